# revision 16
# baseline (speedup 1.0000x reference)
"""KNN classifier layer (B=1024, N=32768, D=64, k=8, C=6) on 8 trn2 cores.

Strategy: shard queries (batch) across the 8 cores, 128 queries per core;
replicate the training set. Per core:
  key[q, n] = x_q . X_n - |X_n|^2/2   (monotone decreasing in distance^2)
computed with fp16 hi/lo splitting so the PE runs at full bf16/fp16 rate
(1 cycle/col) instead of the 4x-slower fp32 path, while keeping ~1e-5
absolute key accuracy (validated exact on the real data; the 8th/9th
neighbor key gap is >= 1.2e-4):
  mm1 (K=66):  [xh; 1; 1] . [Xh; bias_h; bias_l]  = xh.Xh + bias
  mm2 (K=128): [xl; xh]   . [Xh; Xl]              = xl.Xh + xh.Xl
(the dropped xl.Xl term is ~1e-7). Both accumulate in fp32 PSUM.
X_train is pre-sorted by class on the host so each class is a contiguous
column block (padded to a multiple of 8 columns with -30000-keyed dummies).

Single full pass over the keys: DVE max8 per class-pure segment, reading
PSUM directly (no PSUM->SBUF evacuation, no SBUF keys buffer). The label
histogram needs no indices: per class, count of that class's top-8
candidates >= t_q (t_q = global 8th-largest key) equals the number of
top-8 neighbors of that class, because at most 8 keys anywhere are
>= t_q. So after the scan everything is O(1)-sized.
"""

import numpy as np

B, N, D, K, C = 1024, 32768, 64, 8, 6
NCORES = 8
Q = B // NCORES  # queries per core

CHUNK = 2048  # PSUM macro-chunk (4 banks, double-buffered)
SUB = 512     # matmul moving free dim (PSUM bank limit for fp32 output)
NEG = -30000.0  # fp16-safe "never wins" bias for padding columns

_compiled = None
_plan = None


def _plan_layout(y_train: np.ndarray):
    """Class-sort permutation; class blocks padded to multiples of 8 cols;
    chunk/segment schedule."""
    perm = np.argsort(y_train, kind="stable")
    counts = np.bincount(y_train, minlength=C).astype(int)
    w8 = [int(-(-c // 8) * 8) for c in counts]  # pad to multiple of 8
    starts = np.concatenate([[0], np.cumsum(w8)]).astype(int)
    total8 = int(starts[-1])
    # ramp-in with small chunks so the first max8 starts early, then full
    # 2048 chunks, then a ragged tail chunk padded to a multiple of 64
    chunks = [512, 512, 512, 512]
    rem = total8 - sum(chunks)
    chunks += [CHUNK] * (rem // CHUNK)
    rem -= (rem // CHUNK) * CHUNK
    if rem:
        chunks.append(int(-(-rem // 64) * 64))
    tot_cols = sum(chunks)

    # class-pure segments: intersect class regions with chunk windows
    offs = [0]
    for w in chunks:
        offs.append(offs[-1] + w)
    segments = []  # (chunk_idx, off_in_chunk, width, class)
    nseg_per_class = [0] * C
    for m, w in enumerate(chunks):
        c0 = offs[m]
        c1 = c0 + w
        for c in range(C):
            s = max(int(starts[c]), c0)
            e = min(int(starts[c + 1]), c1)
            if e > s:
                segments.append((m, s - c0, e - s, c))
                nseg_per_class[c] += 1
    # candidate-slot layout grouped by class
    cstart = [0] * C
    acc = 0
    for c in range(C):
        cstart[c] = acc
        acc += 8 * nseg_per_class[c]
    n_cand = acc
    return perm, counts, starts, chunks, tot_cols, segments, cstart, n_cand


def _build_nc(plan, finalize: bool = True):
    import concourse.bacc as bacc
    import concourse.mybir as mybir
    from concourse.tile import TileContext

    perm, counts, starts, chunks, tot_cols, segments, cstart, n_cand = plan
    f32 = mybir.dt.float32
    f16 = mybir.dt.float16
    nc = bacc.Bacc(None, target_bir_lowering=False, debug=False)

    lhsT1_d = nc.declare_dram_parameter("lhsT1", [D + 2, Q], f16, isOutput=False)
    lhsT2_d = nc.declare_dram_parameter("lhsT2", [2 * D, Q], f16, isOutput=False)
    t1_d = nc.declare_dram_parameter("t1", [D + 2, tot_cols], f16, isOutput=False)
    t2_d = nc.declare_dram_parameter("t2", [2 * D, tot_cols], f16, isOutput=False)
    out_d = nc.declare_dram_parameter("out", [Q, C], f32, isOutput=True)

    # per-chunk segment lists
    segs_by_chunk = {}
    slot = [0] * C
    for (m, off, w, c) in segments:
        segs_by_chunk.setdefault(m, []).append((off, w, c, slot[c]))
        slot[c] += 1

    # last chunk index in which each class has a segment (for early merges)
    nseg = [0] * C
    last_chunk_of_class = [0] * C
    for (m, _, _, c) in segments:
        nseg[c] += 1
        last_chunk_of_class[c] = max(last_chunk_of_class[c], m)
    merges_after_chunk = {}
    for c in range(C):
        merges_after_chunk.setdefault(last_chunk_of_class[c], []).append(c)

    with TileContext(nc) as tc:
        with (
            tc.tile_pool(name="const", bufs=1) as const_pool,
            tc.tile_pool(name="rhs1", bufs=4) as rhs1_pool,
            tc.tile_pool(name="rhs2", bufs=4) as rhs2_pool,
            tc.tile_pool(name="psum", bufs=2, space="PSUM") as psum_pool,
            tc.tile_pool(name="small", bufs=1) as small_pool,
        ):
            lhsT1_sb = const_pool.tile([D + 2, Q], f16)
            nc.scalar.dma_start(out=lhsT1_sb, in_=lhsT1_d[:, :])
            lhsT2_sb = const_pool.tile([2 * D, Q], f16)
            nc.sync.dma_start(out=lhsT2_sb, in_=lhsT2_d[:, :])

            cand = small_pool.tile([Q, n_cand], f32)
            cls8 = small_pool.tile([Q, C, 8], f32)

            offs = [0]
            for w in chunks:
                offs.append(offs[-1] + w)
            # DMA groups: ramp chunks individually (fast first data), then
            # pairs of chunks per DMA (fewer triggers, bigger packets)
            dma_groups = []
            m = 0
            while m < len(chunks):
                if chunks[m] != CHUNK:
                    dma_groups.append((m, 1))
                    m += 1
                elif m + 1 < len(chunks):
                    dma_groups.append((m, 2))
                    m += 2
                else:
                    dma_groups.append((m, 1))
                    m += 1
            rhs_of_chunk = {}
            for (g0, gn) in dma_groups:
                gw = sum(chunks[g0 : g0 + gn])
                c0 = offs[g0]
                # two trigger queues: t1 on the scalar engine, t2 on sync
                rhs1 = rhs1_pool.tile([D + 2, gw], f16)
                nc.scalar.dma_start(out=rhs1, in_=t1_d[:, c0 : c0 + gw])
                rhs2 = rhs2_pool.tile([2 * D, gw], f16)
                nc.sync.dma_start(out=rhs2, in_=t2_d[:, c0 : c0 + gw])
                for mi in range(gn):
                    rhs_of_chunk[g0 + mi] = (rhs1, rhs2, offs[g0 + mi] - c0)

            for m, w in enumerate(chunks):
                rhs1, rhs2, ro = rhs_of_chunk[m]
                ps = psum_pool.tile([Q, w], f32)
                # same-weight matmuls adjacent to minimize weight reloads
                for j in range(0, w, SUB):
                    sw = min(SUB, w - j)
                    nc.tensor.matmul(
                        ps[:, j : j + sw],
                        lhsT=lhsT1_sb,
                        rhs=rhs1[:, ro + j : ro + j + sw],
                        start=True,
                        stop=False,
                    )
                for j in range(0, w, SUB):
                    sw = min(SUB, w - j)
                    nc.tensor.matmul(
                        ps[:, j : j + sw],
                        lhsT=lhsT2_sb,
                        rhs=rhs2[:, ro + j : ro + j + sw],
                        start=False,
                        stop=True,
                    )
                for (off, sw, c, si) in segs_by_chunk.get(m, []):
                    dst = cstart[c] + 8 * si
                    nc.vector.max(
                        out=cand[:, dst : dst + 8],
                        in_=ps[:, off : off + sw],
                    )
                # merge a class's candidates as soon as its last segment is
                # scanned, hiding the merge under later chunks' work
                for c in merges_after_chunk.get(m, []):
                    nc.vector.max(
                        out=cls8[:, c, :],
                        in_=cand[:, cstart[c] : cstart[c] + 8 * nseg[c]],
                    )
            # global top-8 -> threshold t_q
            g8 = small_pool.tile([Q, 8], f32)
            nc.vector.max(out=g8, in_=cls8)
            tq = g8[:, 7:8]

            # prob[q, c] = 0.125 * #{class-c candidates >= t_q}
            # (counts always sum to exactly 8, so this is the normalized
            # histogram directly)
            scr = small_pool.tile([Q, C, 8], f32)
            nc.vector.tensor_scalar(
                out=scr,
                in0=cls8,
                scalar1=tq,
                scalar2=0.125,
                op0=mybir.AluOpType.is_ge,
                op1=mybir.AluOpType.mult,
            )
            prob = small_pool.tile([Q, C], f32)
            nc.vector.tensor_reduce(
                out=prob,
                in_=scr,
                axis=mybir.AxisListType.X,
                op=mybir.AluOpType.add,
            )
            nc.sync.dma_start(out=out_d[:, :], in_=prob)

    if finalize:
        nc.finalize()
    return nc


def _split16(a: np.ndarray):
    h = a.astype(np.float16)
    l = (a - h.astype(np.float32)).astype(np.float16)
    return h, l


def _prepare(x: np.ndarray, X_train: np.ndarray, y_train: np.ndarray):
    global _plan
    if _plan is None:
        _plan = _plan_layout(y_train)
    perm, counts, starts, chunks, tot_cols, segments, cstart, n_cand = _plan

    Xs = X_train[perm].astype(np.float32)  # [N, D] class-sorted
    t_sq = np.sum(Xs * Xs, axis=1)

    xt = np.zeros((D, tot_cols), dtype=np.float32)
    b = np.full(tot_cols, NEG, dtype=np.float32)  # padding columns never win
    pos = 0
    for c in range(C):
        s = int(starts[c])
        w = int(counts[c])
        xt[:, s : s + w] = Xs[pos : pos + w].T
        b[s : s + w] = -0.5 * t_sq[pos : pos + w]
        pos += w
    Xh, Xl = _split16(xt)
    bh = b.astype(np.float16)
    bl = (b - bh.astype(np.float32)).astype(np.float16)
    t1 = np.concatenate([Xh, bh[None, :], bl[None, :]], axis=0)  # [66, tot]
    t2 = np.concatenate([Xh, Xl], axis=0)  # [128, tot]
    return t1, t2


def _make_in_maps(x: np.ndarray, t1: np.ndarray, t2: np.ndarray):
    in_maps = []
    for core in range(NCORES):
        xc = x[core * Q : (core + 1) * Q].astype(np.float32)  # [Q, D]
        xh, xl = _split16(xc.T)
        lhsT1 = np.concatenate([xh, np.ones((2, Q), np.float16)], axis=0)
        lhsT2 = np.concatenate([xl, xh], axis=0)
        in_maps.append({"lhsT1": lhsT1, "lhsT2": lhsT2, "t1": t1, "t2": t2})
    return in_maps


def _run(x, X_train, y_train, trace=False, tmpdir=None):
    global _compiled
    from concourse.bass_utils import run_bass_kernel_spmd

    t1, t2 = _prepare(x, X_train, y_train)
    if _compiled is None:
        _compiled = _build_nc(_plan)
    res = run_bass_kernel_spmd(
        _compiled,
        _make_in_maps(x, t1, t2),
        core_ids=list(range(NCORES)),
        trace=trace,
        tmpdir=tmpdir,
    )
    out = np.concatenate([res.results[i]["out"] for i in range(NCORES)], axis=0)
    return out.astype(np.float32), res


def kernel(x: np.ndarray, X_train: np.ndarray, y_train: np.ndarray) -> np.ndarray:
    out, _ = _run(x, X_train, y_train)
    return out
